# revision 42
# baseline (speedup 1.0000x reference)
"""Trainium2 Bass kernel for DigitCaps dynamic-routing layer (v6, bf16).

priors[c,b,n,o] = sum_i x[b,n,i] * W[c,n,i,o]; 3 softmax-routing iterations.
Output: squash(sum_n probs * priors) of the last iteration, [C,B,1,DOUT].

Strategy (B data-parallel over 8 cores, BL=32 per core, all on-chip, bf16):
  - it0 (uniform probs): s0^T[b,(c,o)] via x-stationary matmuls (160 cols per
    (n,i) chunk), then 5 capsule-pair transposes into the squash layout.
  - it1/2: s[(c,o),(c,b)] block-diag via ws-stationary matmuls, exp-weights
    folded into x (xp bf16); moving columns split 256/64 between the two
    (c,o)-row groups.
  - squash in a capsule-pair layout scr2[32, 320]: aligned scalar-engine
    copies of [32,64] pair blocks from PSUM (junk killed by a constant
    mask); norms AND Z reduced+replicated by all-ones bf16 matmuls;
    divide-free scale g = sqrt(qs)/(Z^2 + qs); v = s*g. The zeroed pair
    blocks double as the block-diag V operand of the next agreement phase.
  - agreement a[c,b,n] = sum_i x * (W v): Wv on PE into 2-bank PSUM pair
    tiles, x-mult as i-pair-wide DVE ops against a host-replicated xrep,
    i-reduction as a bf16 tree split across DVE and GpSimd; Z accumulated
    per n-block by ones-matmuls into PSUM.
  - a(k) and s(k+1) software-pipelined per n-block (skew) to keep PE fed;
    priors are never materialized.

Engine partition bases: matmul operands and multi-input DVE/Pool ops at
base 0; single-input scalar.copy moves between 32-aligned bases. GpSimd
never touches PSUM. Layouts (i-major): chunk ch in [0,72): i = ch//9,
nb = ch%9, partition = n%128. (c,b) packed cb = 32*c + b.
"""

import numpy as np
import ml_dtypes

C, N, DIN, DOUT, B = 10, 1152, 8, 16, 256
NCORES, BL = 8, B // 8
NI = N * DIN          # 9216
NB = N // 128         # 9
NCH = DIN * NB        # 72
TW = C * BL           # 320
CD = C * DOUT         # 160

_PROG = None


def _build_program(stage=6):
    import concourse.bacc as bacc
    import concourse.tile as tile
    from concourse import mybir

    f32 = mybir.dt.float32
    bf16 = mybir.dt.bfloat16
    OP = mybir.AluOpType
    AF = mybir.ActivationFunctionType

    nc = bacc.Bacc("TRN2", target_bir_lowering=False, debug=False,
                   enable_asserts=False, num_devices=NCORES)

    WSP = 8
    wpc = (NCH // WSP) * CD
    xin_d = nc.dram_tensor("xin", [128, NCH * BL], bf16,
                           kind="ExternalInput").ap()
    xrep_d = nc.dram_tensor("xrep", [128, NCH * TW], bf16,
                            kind="ExternalInput").ap()
    ws_d = nc.dram_tensor("ws", [128, NCH * CD], bf16,
                          kind="ExternalInput").ap()
    wt4a_d = nc.dram_tensor("wt4a", [128, NI], bf16,
                            kind="ExternalInput").ap()
    wt4b_d = nc.dram_tensor("wt4b", [32, NI], bf16,
                            kind="ExternalInput").ap()
    ident_d = nc.dram_tensor("ident", [64, 64], f32,
                             kind="ExternalInput").ap()
    mask_d = nc.dram_tensor("mask", [32, TW], f32,
                            kind="ExternalInput").ap()
    vout_d = nc.dram_tensor("vout", [32, TW], f32,
                            kind="ExternalOutput").ap()

    with tile.TileContext(nc) as tc:
        with (
            tc.tile_pool(name="const", bufs=1) as cp,
            tc.tile_pool(name="xpb", bufs=6) as xpp,
            tc.tile_pool(name="tsl", bufs=3) as tpp,
            tc.tile_pool(name="vt", bufs=2) as vp,
            tc.tile_pool(name="pssa", bufs=1, space="PSUM") as pssa,
            tc.tile_pool(name="pssb", bufs=1, space="PSUM") as pssb,
            tc.tile_pool(name="psw", bufs=5, space="PSUM") as psw,
            tc.tile_pool(name="pzr", bufs=1, space="PSUM") as pzr,
        ):
            x_sb = cp.tile([128, NCH * BL], bf16)
            xrep = cp.tile([128, NCH * TW], bf16)
            wsp = [cp.tile([128, wpc], bf16, tag=f"ws{p}", name=f"ws{p}")
                   for p in range(WSP)]
            wt4a = cp.tile([128, NI], bf16)
            wt4b = cp.tile([32, NI], bf16)
            ident = cp.tile([64, 64], f32)
            mask = cp.tile([32, TW], f32)
            etb = cp.tile([128, NB * TW], bf16)
            Lbb = cp.tile([128, NB * TW], bf16)
            bdV01 = cp.tile([128, 256], bf16)
            bdV2 = cp.tile([32, 64], bf16)
            ones128 = cp.tile([128, 32], bf16)
            ones32 = cp.tile([32, 32], bf16)

            nc.sync.dma_start(x_sb[:], xin_d[:])
            for p_ in range(WSP):
                nc.sync.dma_start(wsp[p_][:], ws_d[:, wpc * p_:wpc * (p_ + 1)])
            nc.sync.dma_start(wt4b[:], wt4b_d[:])
            for h in range(2):
                nc.sync.dma_start(wt4a[:, NI // 2 * h:NI // 2 * (h + 1)],
                                  wt4a_d[:, NI // 2 * h:NI // 2 * (h + 1)])
            nc.sync.dma_start(ident[:], ident_d[:])
            nc.sync.dma_start(mask[:], mask_d[:])
            # xrep is nb-major: slab nb = [128, DIN*TW] feeds a_block(nb)
            xsl = DIN * TW
            for nb_ in range(NB):
                nc.sync.dma_start(xrep[:, xsl * nb_:xsl * (nb_ + 1)],
                                  xrep_d[:, xsl * nb_:xsl * (nb_ + 1)])
            nc.vector.memset(bdV01[:].bitcast(mybir.dt.uint32), 0)
            nc.vector.memset(bdV2[:].bitcast(mybir.dt.uint32), 0)
            nc.vector.memset(ones128[:].bitcast(mybir.dt.uint32), 0x3F803F80)
            nc.vector.memset(ones32[:].bitcast(mybir.dt.uint32), 0x3F803F80)

            def ws_chunk(ch, a, b):
                p_ = ch // (NCH // WSP)
                base = CD * (ch - p_ * (NCH // WSP))
                return wsp[p_][:, base + a:base + b]

            # scr2 pair layout [32, TW]: pair p (capsules 2p, 2p+1) occupies
            # cols 64p:64p+64; rows 0:16 <-> c=2p (block cols 0:32),
            # rows 16:32 <-> c=2p+1 (block cols 32:64). mask zeroes the rest.
            def squash(it, fill_scr2, zinfo, need_bdv, final):
                scr2 = vp.tile([32, TW], f32, tag="scr2")
                fill_scr2(scr2)
                s2m = vp.tile([32, TW], f32, tag="s2m")
                nc.vector.tensor_tensor(out=s2m[:], in0=scr2[:], in1=mask[:],
                                        op=OP.mult)
                sq = vp.tile([32, TW], bf16, tag="sq")
                nc.vector.tensor_tensor(out=sq[:], in0=s2m[:], in1=scr2[:],
                                        op=OP.mult)
                # norms: reduce over 32 partitions AND replicate to all rows
                # in one all-ones matmul.
                qsr = psw.tile([32, TW], f32, tag="wv", name=f"qsr{it}")
                nc.tensor.matmul(qsr[:], ones32[:], sq[:],
                                 start=True, stop=True)
                d = vp.tile([32, TW], f32, tag="d")
                rt = vp.tile([32, TW], f32, tag="rt")
                rec = vp.tile([32, TW], f32, tag="rec")
                g = vp.tile([32, TW], f32, tag="g")
                # g = sqrt(qs) / (Z^2 + qs); it0: Z = N.
                if it == 0:
                    nc.vector.tensor_scalar_add(d[:], qsr[:], float(N) ** 2)
                else:
                    nc.vector.tensor_tensor(out=d[:], in0=qsr[:],
                                            in1=zinfo["Z2"][:], op=OP.add)
                nc.scalar.activation(rt[:], qsr[:], AF.Sqrt)
                nc.vector.reciprocal_approx_fast(rec[:], d[:])
                nc.gpsimd.tensor_tensor(out=g[:], in0=rt[:], in1=rec[:],
                                        op=OP.mult)
                v2 = vp.tile([32, TW], f32, tag="v2")
                nc.vector.tensor_tensor(out=v2[:], in0=s2m[:], in1=g[:],
                                        op=OP.mult)
                if need_bdv:
                    for p_ in range(4):
                        nc.scalar.copy(
                            bdV01[32 * p_:32 * (p_ + 1),
                                  64 * p_:64 * (p_ + 1)],
                            v2[:, 64 * p_:64 * (p_ + 1)])
                    nc.scalar.copy(bdV2[:], v2[:, 256:TW])
                if final:
                    nc.sync.dma_start(vout_d[:], v2[:])

            # -------- it0: uniform probs, x-stationary transposed matmul ----
            psT = psw.tile([BL, CD], f32, tag="wv", name="psT")
            for ch in range(NCH):
                nc.tensor.matmul(psT[:], x_sb[:, BL * ch:BL * (ch + 1)],
                                 ws_chunk(ch, 0, CD),
                                 start=(ch == 0), stop=(ch == NCH - 1))
            sT2 = vp.tile([64, CD], f32, tag="sT2")
            nc.scalar.copy(sT2[0:BL, :], psT[:])
            nc.scalar.copy(sT2[BL:64, :], psT[:])
            pscr2 = psw.tile([32, TW], f32, tag="wv", name="pscr2")
            for p_ in range(5):
                nc.tensor.transpose(pscr2[0:32, 64 * p_:64 * (p_ + 1)],
                                    sT2[:, 32 * p_:32 * (p_ + 1)], ident[:])

            def fill0(scr2):
                nc.scalar.copy(scr2[:], pscr2[:])

            squash(0, fill0, None, need_bdv=True, final=(stage < 4))

            # -------- rounds: a(k) + exp + Z + s(k+1), pipelined ------------
            def round_(k, nbmax=NB):
                psa = pssa.tile([128, 8 * BL], f32, tag="psa")
                psb = pssb.tile([32, 2 * BL], f32, tag="psb")
                Zr = pzr.tile([32, TW], f32, tag="zr")
                xpbs = {}
                T1, T2, T4 = TW, 2 * TW, 4 * TW

                def gp_add(out, in0, in1):
                    nc.gpsimd.tensor_tensor(out=out, in0=in0, in1=in1,
                                            op=OP.add)

                def a_block(nb):
                    tsl = tpp.tile([128, DIN * TW], bf16, tag="t")
                    for i in range(DIN):
                        ch = i * NB + nb
                        pwv = psw.tile([128, TW], f32, tag="wv")
                        nc.tensor.matmul(pwv[:, 0:256],
                                         wt4a[:, 128 * ch:128 * (ch + 1)],
                                         bdV01[:], start=True, stop=True)
                        nc.tensor.matmul(pwv[:, 256:320],
                                         wt4b[0:32, 128 * ch:128 * (ch + 1)],
                                         bdV2[:], start=True, stop=True)
                        nc.vector.tensor_tensor(
                            out=tsl[:, TW * i:TW * (i + 1)],
                            in0=pwv[:],
                            in1=xrep[:, TW * (nb * DIN + i):
                                     TW * (nb * DIN + i) + TW],
                            op=OP.mult)
                    nc.vector.tensor_tensor(out=tsl[:, 0:T4],
                                            in0=tsl[:, 0:T4],
                                            in1=tsl[:, T4:2 * T4], op=OP.add)
                    gp_add(tsl[:, 0:T2], tsl[:, 0:T2], tsl[:, T2:T4])
                    Ls = Lbb[:, TW * nb:TW * (nb + 1)]
                    es = etb[:, TW * nb:TW * (nb + 1)]
                    if k == 0:
                        gp_add(Ls, tsl[:, 0:T1], tsl[:, T1:T2])
                    else:
                        gp_add(tsl[:, 0:T1], tsl[:, 0:T1], tsl[:, T1:T2])
                        gp_add(Ls, Ls, tsl[:, 0:T1])
                    nc.scalar.activation(es, Ls, AF.Exp)
                    nc.tensor.matmul(Zr[:], ones128[:], es,
                                     start=(nb == 0), stop=(nb == nbmax - 1))

                def xp_block(nb):
                    es = etb[:, TW * nb:TW * (nb + 1)]
                    xpb = xpp.tile([128, DIN * TW], bf16, tag="xp")
                    for i in range(DIN):
                        eng = nc.vector if i < 2 else nc.gpsimd
                        eng.tensor_tensor(
                            out=xpb[:, TW * i:TW * (i + 1)],
                            in0=xrep[:, TW * (nb * DIN + i):
                                     TW * (nb * DIN + i) + TW],
                            in1=es, op=OP.mult)
                    xpbs[nb] = xpb

                def s_block(nb):
                    xpb = xpbs[nb]
                    for i in range(DIN):
                        ch = i * NB + nb
                        first = (nb == 0 and i == 0)
                        last = (nb == NB - 1 and i == DIN - 1)
                        nc.tensor.matmul(psa[:], ws_chunk(ch, 0, 128),
                                         xpb[:, TW * i:TW * i + 256],
                                         start=first, stop=last)
                        nc.tensor.matmul(psb[:], ws_chunk(ch, 128, CD),
                                         xpb[:, TW * i + 256:TW * i + 320],
                                         start=first, stop=last)

                SKEW = 2
                for step in range(NB + SKEW):
                    if step < nbmax:
                        a_block(step)
                    if 1 <= step < nbmax + 1:
                        xp_block(step - 1)
                    if SKEW <= step < nbmax + SKEW:
                        s_block(step - SKEW)

                Z2 = vp.tile([32, TW], f32, tag="Z2")
                nc.scalar.activation(Z2[:], Zr[:], AF.Square)

                def fill(scr2):
                    for p_ in range(4):
                        nc.scalar.copy(scr2[:, 64 * p_:64 * (p_ + 1)],
                                       psa[32 * p_:32 * (p_ + 1),
                                           64 * p_:64 * (p_ + 1)])
                    nc.scalar.copy(scr2[:, 256:TW], psb[:])

                return {"Z2": Z2}, fill

            if stage >= 4:
                z1, fill1 = round_(0, nbmax=(NB if stage >= 5 else 1))
                if stage >= 6:
                    squash(1, fill1, z1, need_bdv=True, final=False)
                    z2, fill2 = round_(1)
                    squash(2, fill2, z2, need_bdv=False, final=True)
                else:
                    squash(1, fill1, z1, need_bdv=False, final=True)

    nc.compile()
    return nc


def _get_prog():
    global _PROG
    if _PROG is None:
        _PROG = _build_program()
    return _PROG


def _host_inputs(x, W):
    bf = ml_dtypes.bfloat16
    xf = np.ascontiguousarray(x, dtype=np.float32)
    Wf = np.ascontiguousarray(W, dtype=np.float32)
    ws = (Wf.transpose(2, 1, 0, 3)
          .reshape(DIN, NB, 128, C, DOUT)
          .transpose(2, 0, 1, 3, 4)
          .reshape(128, NCH * CD)).astype(bf)
    a4 = Wf.transpose(0, 3, 2, 1).reshape(C, DOUT, NI)    # [c, o, (i, n)]
    wt4a = np.concatenate(
        [a4[0:4].reshape(64, NI), a4[4:8].reshape(64, NI)], axis=0).astype(bf)
    wt4b = a4[8:10].reshape(32, NI).astype(bf)
    ident = np.eye(64, dtype=np.float32)
    mask = np.zeros((32, TW), dtype=np.float32)
    for col in range(TW):
        h = (col // 32) % 2
        mask[16 * h:16 * (h + 1), col] = 1.0
    maps = []
    for k in range(NCORES):
        xs = (xf[BL * k:BL * (k + 1)]
              .transpose(2, 1, 0)
              .reshape(DIN, NB, 128, BL)
              .transpose(2, 0, 1, 3)
              .reshape(128, NCH * BL)).astype(bf)
        # xrep: [p, (nb, i, c, b)] = xs broadcast over c, nb-major
        xr = (np.broadcast_to(xs.reshape(128, DIN, NB, 1, BL),
                              (128, DIN, NB, C, BL))
              .transpose(0, 2, 1, 3, 4).reshape(128, NCH * TW))
        maps.append({
            "xin": np.ascontiguousarray(xs),
            "xrep": np.ascontiguousarray(xr),
            "ws": np.ascontiguousarray(ws),
            "wt4a": np.ascontiguousarray(wt4a),
            "wt4b": np.ascontiguousarray(wt4b),
            "ident": ident,
            "mask": mask,
        })
    return maps


def kernel(x, W):
    from concourse.bass_utils import run_bass_kernel_spmd
    nc = _get_prog()
    in_maps = _host_inputs(x, W)
    res = run_bass_kernel_spmd(nc, in_maps, core_ids=list(range(NCORES)))
    out = np.zeros((C, B, 1, DOUT), dtype=np.float32)
    for k in range(NCORES):
        vo = res.results[k]["vout"]              # [32, TW] pair layout
        for c_ in range(C):
            p_, h = c_ // 2, c_ % 2
            blk = vo[16 * h:16 * (h + 1),
                     64 * p_ + 32 * h:64 * p_ + 32 * (h + 1)]
            out[c_, BL * k:BL * (k + 1), 0, :] = blk.T
    return out


# revision 43
# speedup vs baseline: 1.0337x; 1.0337x over previous
"""Trainium2 Bass kernel for DigitCaps dynamic-routing layer (v6, bf16).

priors[c,b,n,o] = sum_i x[b,n,i] * W[c,n,i,o]; 3 softmax-routing iterations.
Output: squash(sum_n probs * priors) of the last iteration, [C,B,1,DOUT].

Strategy (B data-parallel over 8 cores, BL=32 per core, all on-chip, bf16):
  - it0 (uniform probs): s0^T[b,(c,o)] via x-stationary matmuls (160 cols per
    (n,i) chunk), then 5 capsule-pair transposes into the squash layout.
  - it1/2: s[(c,o),(c,b)] block-diag via ws-stationary matmuls, exp-weights
    folded into x (xp bf16); moving columns split 256/64 between the two
    (c,o)-row groups.
  - squash in a capsule-pair layout scr2[32, 320]: aligned scalar-engine
    copies of [32,64] pair blocks from PSUM (junk killed by a constant
    mask); norms AND Z reduced+replicated by all-ones bf16 matmuls;
    divide-free scale g = sqrt(qs)/(Z^2 + qs); v = s*g. The zeroed pair
    blocks double as the block-diag V operand of the next agreement phase.
  - agreement a[c,b,n] = sum_i x * (W v): Wv on PE into 2-bank PSUM pair
    tiles, x-mult as i-pair-wide DVE ops against a host-replicated xrep,
    i-reduction as a bf16 tree split across DVE and GpSimd; Z accumulated
    per n-block by ones-matmuls into PSUM.
  - a(k) and s(k+1) software-pipelined per n-block (skew) to keep PE fed;
    priors are never materialized.

Engine partition bases: matmul operands and multi-input DVE/Pool ops at
base 0; single-input scalar.copy moves between 32-aligned bases. GpSimd
never touches PSUM. Layouts (i-major): chunk ch in [0,72): i = ch//9,
nb = ch%9, partition = n%128. (c,b) packed cb = 32*c + b.
"""

import numpy as np
import ml_dtypes

C, N, DIN, DOUT, B = 10, 1152, 8, 16, 256
NCORES, BL = 8, B // 8
NI = N * DIN          # 9216
NB = N // 128         # 9
NCH = DIN * NB        # 72
TW = C * BL           # 320
CD = C * DOUT         # 160

_PROG = None


def _build_program(stage=6):
    import concourse.bacc as bacc
    import concourse.tile as tile
    from concourse import mybir

    f32 = mybir.dt.float32
    bf16 = mybir.dt.bfloat16
    OP = mybir.AluOpType
    AF = mybir.ActivationFunctionType

    nc = bacc.Bacc("TRN2", target_bir_lowering=False, debug=False,
                   enable_asserts=False, num_devices=NCORES)

    WSP = 8
    wpc = (NCH // WSP) * CD
    xin_d = nc.dram_tensor("xin", [128, NCH * BL], bf16,
                           kind="ExternalInput").ap()
    xrep_d = nc.dram_tensor("xrep", [128, NCH * TW], bf16,
                            kind="ExternalInput").ap()
    ws_d = nc.dram_tensor("ws", [128, NCH * CD], bf16,
                          kind="ExternalInput").ap()
    wt4a_d = nc.dram_tensor("wt4a", [128, NI], bf16,
                            kind="ExternalInput").ap()
    wt4b_d = nc.dram_tensor("wt4b", [32, NI], bf16,
                            kind="ExternalInput").ap()
    ident_d = nc.dram_tensor("ident", [64, 64], f32,
                             kind="ExternalInput").ap()
    mask_d = nc.dram_tensor("mask", [32, TW], f32,
                            kind="ExternalInput").ap()
    vout_d = nc.dram_tensor("vout", [32, TW], f32,
                            kind="ExternalOutput").ap()

    with tile.TileContext(nc) as tc:
        with (
            tc.tile_pool(name="const", bufs=1) as cp,
            tc.tile_pool(name="xpb", bufs=6) as xpp,
            tc.tile_pool(name="tsl", bufs=3) as tpp,
            tc.tile_pool(name="vt", bufs=2) as vp,
            tc.tile_pool(name="pssa", bufs=1, space="PSUM") as pssa,
            tc.tile_pool(name="pssb", bufs=1, space="PSUM") as pssb,
            tc.tile_pool(name="psw", bufs=5, space="PSUM") as psw,
            tc.tile_pool(name="pzr", bufs=1, space="PSUM") as pzr,
        ):
            x_sb = cp.tile([128, NCH * BL], bf16)
            xrep = cp.tile([128, NCH * TW], bf16)
            wsp = [cp.tile([128, wpc], bf16, tag=f"ws{p}", name=f"ws{p}")
                   for p in range(WSP)]
            wt4a = cp.tile([128, NI], bf16)
            wt4b = cp.tile([32, NI], bf16)
            ident = cp.tile([64, 64], f32)
            mask = cp.tile([32, TW], f32)
            etb = cp.tile([128, NB * TW], bf16)
            Lbb = cp.tile([128, NB * TW], bf16)
            bdV01 = cp.tile([128, 256], bf16)
            bdV2 = cp.tile([32, 64], bf16)
            ones128 = cp.tile([128, 32], bf16)
            ones32 = cp.tile([32, 32], bf16)

            nc.sync.dma_start(x_sb[:], xin_d[:])
            for p_ in range(WSP):
                nc.sync.dma_start(wsp[p_][:], ws_d[:, wpc * p_:wpc * (p_ + 1)])
            nc.sync.dma_start(wt4b[:], wt4b_d[:])
            for h in range(2):
                nc.sync.dma_start(wt4a[:, NI // 2 * h:NI // 2 * (h + 1)],
                                  wt4a_d[:, NI // 2 * h:NI // 2 * (h + 1)])
            nc.sync.dma_start(ident[:], ident_d[:])
            nc.sync.dma_start(mask[:], mask_d[:])
            # xrep is nb-major: slab nb = [128, DIN*TW] feeds a_block(nb)
            xsl = DIN * TW
            for nb_ in range(NB):
                nc.sync.dma_start(xrep[:, xsl * nb_:xsl * (nb_ + 1)],
                                  xrep_d[:, xsl * nb_:xsl * (nb_ + 1)])
            nc.vector.memset(bdV01[:].bitcast(mybir.dt.uint32), 0)
            nc.vector.memset(bdV2[:].bitcast(mybir.dt.uint32), 0)
            nc.vector.memset(ones128[:].bitcast(mybir.dt.uint32), 0x3F803F80)
            nc.vector.memset(ones32[:].bitcast(mybir.dt.uint32), 0x3F803F80)

            def ws_chunk(ch, a, b):
                p_ = ch // (NCH // WSP)
                base = CD * (ch - p_ * (NCH // WSP))
                return wsp[p_][:, base + a:base + b]

            # scr2 pair layout [32, TW]: pair p (capsules 2p, 2p+1) occupies
            # cols 64p:64p+64; rows 0:16 <-> c=2p (block cols 0:32),
            # rows 16:32 <-> c=2p+1 (block cols 32:64). mask zeroes the rest.
            def squash(it, fill_scr2, zinfo, need_bdv, final):
                scr2 = vp.tile([32, TW], f32, tag="scr2")
                fill_scr2(scr2)
                s2m = vp.tile([32, TW], f32, tag="s2m")
                nc.vector.tensor_tensor(out=s2m[:], in0=scr2[:], in1=mask[:],
                                        op=OP.mult)
                sq = vp.tile([32, TW], bf16, tag="sq")
                nc.vector.tensor_tensor(out=sq[:], in0=s2m[:], in1=scr2[:],
                                        op=OP.mult)
                # norms: reduce over 32 partitions AND replicate to all rows
                # in one all-ones matmul.
                qsr = psw.tile([32, TW], f32, tag="wv", name=f"qsr{it}")
                nc.tensor.matmul(qsr[:], ones32[:], sq[:],
                                 start=True, stop=True)
                d = vp.tile([32, TW], f32, tag="d")
                rt = vp.tile([32, TW], f32, tag="rt")
                rec = vp.tile([32, TW], f32, tag="rec")
                g = vp.tile([32, TW], f32, tag="g")
                # g = sqrt(qs) / (Z^2 + qs); it0: Z = N.
                if it == 0:
                    nc.vector.tensor_scalar_add(d[:], qsr[:], float(N) ** 2)
                else:
                    nc.vector.tensor_tensor(out=d[:], in0=qsr[:],
                                            in1=zinfo["Z2"][:], op=OP.add)
                nc.scalar.activation(rt[:], qsr[:], AF.Sqrt)
                nc.vector.reciprocal_approx_fast(rec[:], d[:])
                nc.gpsimd.tensor_tensor(out=g[:], in0=rt[:], in1=rec[:],
                                        op=OP.mult)
                v2 = vp.tile([32, TW], f32, tag="v2")
                nc.vector.tensor_tensor(out=v2[:], in0=s2m[:], in1=g[:],
                                        op=OP.mult)
                if need_bdv:
                    for p_ in range(4):
                        nc.scalar.copy(
                            bdV01[32 * p_:32 * (p_ + 1),
                                  64 * p_:64 * (p_ + 1)],
                            v2[:, 64 * p_:64 * (p_ + 1)])
                    nc.scalar.copy(bdV2[:], v2[:, 256:TW])
                if final:
                    nc.sync.dma_start(vout_d[:], v2[:])

            # -------- it0: uniform probs, x-stationary transposed matmul ----
            psT = psw.tile([BL, CD], f32, tag="wv", name="psT")
            for ch in range(NCH):
                nc.tensor.matmul(psT[:], x_sb[:, BL * ch:BL * (ch + 1)],
                                 ws_chunk(ch, 0, CD),
                                 start=(ch == 0), stop=(ch == NCH - 1))
            sT2 = vp.tile([64, CD], f32, tag="sT2")
            nc.scalar.copy(sT2[0:BL, :], psT[:])
            nc.scalar.copy(sT2[BL:64, :], psT[:])
            pscr2 = psw.tile([32, TW], f32, tag="wv", name="pscr2")
            for p_ in range(5):
                nc.tensor.transpose(pscr2[0:32, 64 * p_:64 * (p_ + 1)],
                                    sT2[:, 32 * p_:32 * (p_ + 1)], ident[:])

            def fill0(scr2):
                nc.scalar.copy(scr2[:], pscr2[:])

            squash(0, fill0, None, need_bdv=True, final=(stage < 4))

            # -------- rounds: a(k) + exp + Z + s(k+1), pipelined ------------
            def round_(k, nbmax=NB):
                psa = pssa.tile([128, 8 * BL], f32, tag="psa")
                psb = pssb.tile([32, 2 * BL], f32, tag="psb")
                Zr = pzr.tile([32, TW], f32, tag="zr")
                xpbs = {}
                T1, T2, T4 = TW, 2 * TW, 4 * TW

                def gp_add(out, in0, in1):
                    nc.gpsimd.tensor_tensor(out=out, in0=in0, in1=in1,
                                            op=OP.add)

                def a_block(nb):
                    tsl = tpp.tile([128, DIN * TW], bf16, tag="t")
                    for i in range(DIN):
                        ch = i * NB + nb
                        pwv = psw.tile([128, TW], f32, tag="wv")
                        nc.tensor.matmul(pwv[:, 0:256],
                                         wt4a[:, 128 * ch:128 * (ch + 1)],
                                         bdV01[:], start=True, stop=True)
                        nc.tensor.matmul(pwv[:, 256:320],
                                         wt4b[0:32, 128 * ch:128 * (ch + 1)],
                                         bdV2[:], start=True, stop=True)
                        nc.vector.tensor_tensor(
                            out=tsl[:, TW * i:TW * (i + 1)],
                            in0=pwv[:],
                            in1=xrep[:, TW * (nb * DIN + i):
                                     TW * (nb * DIN + i) + TW],
                            op=OP.mult)
                    nc.vector.tensor_tensor(out=tsl[:, 0:T4],
                                            in0=tsl[:, 0:T4],
                                            in1=tsl[:, T4:2 * T4], op=OP.add)
                    gp_add(tsl[:, 0:T2], tsl[:, 0:T2], tsl[:, T2:T4])
                    Ls = Lbb[:, TW * nb:TW * (nb + 1)]
                    es = etb[:, TW * nb:TW * (nb + 1)]
                    if k == 0:
                        gp_add(Ls, tsl[:, 0:T1], tsl[:, T1:T2])
                    else:
                        gp_add(tsl[:, 0:T1], tsl[:, 0:T1], tsl[:, T1:T2])
                        gp_add(Ls, Ls, tsl[:, 0:T1])
                    nc.scalar.activation(es, Ls, AF.Exp)
                    nc.tensor.matmul(Zr[:], ones128[:], es,
                                     start=(nb == 0), stop=(nb == nbmax - 1))

                def xp_block(nb):
                    es = etb[:, TW * nb:TW * (nb + 1)]
                    xpb = xpp.tile([128, DIN * TW], bf16, tag="xp")
                    for i in range(DIN):
                        eng = nc.vector if i < 3 else nc.gpsimd
                        eng.tensor_tensor(
                            out=xpb[:, TW * i:TW * (i + 1)],
                            in0=xrep[:, TW * (nb * DIN + i):
                                     TW * (nb * DIN + i) + TW],
                            in1=es, op=OP.mult)
                    xpbs[nb] = xpb

                def s_block(nb):
                    xpb = xpbs[nb]
                    for i in range(DIN):
                        ch = i * NB + nb
                        first = (nb == 0 and i == 0)
                        last = (nb == NB - 1 and i == DIN - 1)
                        nc.tensor.matmul(psa[:], ws_chunk(ch, 0, 128),
                                         xpb[:, TW * i:TW * i + 256],
                                         start=first, stop=last)
                        nc.tensor.matmul(psb[:], ws_chunk(ch, 128, CD),
                                         xpb[:, TW * i + 256:TW * i + 320],
                                         start=first, stop=last)

                SKEW = 2
                for step in range(NB + SKEW):
                    if step < nbmax:
                        a_block(step)
                    if 1 <= step < nbmax + 1:
                        xp_block(step - 1)
                    if SKEW <= step < nbmax + SKEW:
                        s_block(step - SKEW)

                Z2 = vp.tile([32, TW], f32, tag="Z2")
                nc.scalar.activation(Z2[:], Zr[:], AF.Square)

                def fill(scr2):
                    for p_ in range(4):
                        nc.scalar.copy(scr2[:, 64 * p_:64 * (p_ + 1)],
                                       psa[32 * p_:32 * (p_ + 1),
                                           64 * p_:64 * (p_ + 1)])
                    nc.scalar.copy(scr2[:, 256:TW], psb[:])

                return {"Z2": Z2}, fill

            if stage >= 4:
                z1, fill1 = round_(0, nbmax=(NB if stage >= 5 else 1))
                if stage >= 6:
                    squash(1, fill1, z1, need_bdv=True, final=False)
                    z2, fill2 = round_(1)
                    squash(2, fill2, z2, need_bdv=False, final=True)
                else:
                    squash(1, fill1, z1, need_bdv=False, final=True)

    nc.compile()
    return nc


def _get_prog():
    global _PROG
    if _PROG is None:
        _PROG = _build_program()
    return _PROG


def _host_inputs(x, W):
    bf = ml_dtypes.bfloat16
    xf = np.ascontiguousarray(x, dtype=np.float32)
    Wf = np.ascontiguousarray(W, dtype=np.float32)
    ws = (Wf.transpose(2, 1, 0, 3)
          .reshape(DIN, NB, 128, C, DOUT)
          .transpose(2, 0, 1, 3, 4)
          .reshape(128, NCH * CD)).astype(bf)
    a4 = Wf.transpose(0, 3, 2, 1).reshape(C, DOUT, NI)    # [c, o, (i, n)]
    wt4a = np.concatenate(
        [a4[0:4].reshape(64, NI), a4[4:8].reshape(64, NI)], axis=0).astype(bf)
    wt4b = a4[8:10].reshape(32, NI).astype(bf)
    ident = np.eye(64, dtype=np.float32)
    mask = np.zeros((32, TW), dtype=np.float32)
    for col in range(TW):
        h = (col // 32) % 2
        mask[16 * h:16 * (h + 1), col] = 1.0
    maps = []
    for k in range(NCORES):
        xs = (xf[BL * k:BL * (k + 1)]
              .transpose(2, 1, 0)
              .reshape(DIN, NB, 128, BL)
              .transpose(2, 0, 1, 3)
              .reshape(128, NCH * BL)).astype(bf)
        # xrep: [p, (nb, i, c, b)] = xs broadcast over c, nb-major
        xr = (np.broadcast_to(xs.reshape(128, DIN, NB, 1, BL),
                              (128, DIN, NB, C, BL))
              .transpose(0, 2, 1, 3, 4).reshape(128, NCH * TW))
        maps.append({
            "xin": np.ascontiguousarray(xs),
            "xrep": np.ascontiguousarray(xr),
            "ws": np.ascontiguousarray(ws),
            "wt4a": np.ascontiguousarray(wt4a),
            "wt4b": np.ascontiguousarray(wt4b),
            "ident": ident,
            "mask": mask,
        })
    return maps


def kernel(x, W):
    from concourse.bass_utils import run_bass_kernel_spmd
    nc = _get_prog()
    in_maps = _host_inputs(x, W)
    res = run_bass_kernel_spmd(nc, in_maps, core_ids=list(range(NCORES)))
    out = np.zeros((C, B, 1, DOUT), dtype=np.float32)
    for k in range(NCORES):
        vo = res.results[k]["vout"]              # [32, TW] pair layout
        for c_ in range(C):
            p_, h = c_ // 2, c_ % 2
            blk = vo[16 * h:16 * (h + 1),
                     64 * p_ + 32 * h:64 * p_ + 32 * (h + 1)]
            out[c_, BL * k:BL * (k + 1), 0, :] = blk.T
    return out


# revision 45
# speedup vs baseline: 1.2183x; 1.1786x over previous
"""Trainium2 Bass kernel for DigitCaps dynamic-routing layer (v6, bf16).

priors[c,b,n,o] = sum_i x[b,n,i] * W[c,n,i,o]; 3 softmax-routing iterations.
Output: squash(sum_n probs * priors) of the last iteration, [C,B,1,DOUT].

Strategy (B data-parallel over 8 cores, BL=32 per core, all on-chip, bf16):
  - it0 (uniform probs): s0^T[b,(c,o)] via x-stationary matmuls (160 cols per
    (n,i) chunk), then 5 capsule-pair transposes into the squash layout.
  - it1/2: s[(c,o),(c,b)] block-diag via ws-stationary matmuls, exp-weights
    folded into x (xp bf16); moving columns split 256/64 between the two
    (c,o)-row groups.
  - squash in a capsule-pair layout scr2[32, 320]: aligned scalar-engine
    copies of [32,64] pair blocks from PSUM (junk killed by a constant
    mask); norms AND Z reduced+replicated by all-ones bf16 matmuls;
    divide-free scale g = sqrt(qs)/(Z^2 + qs); v = s*g. The zeroed pair
    blocks double as the block-diag V operand of the next agreement phase.
  - agreement a[c,b,n] = sum_i x * (W v): Wv on PE, x-mult as flat DVE ops
    against a host-replicated nb-major xrep (no broadcast views),
    i-reduction as a bf16 tree split across DVE and GpSimd; Z accumulated
    per n-block by ones-matmuls into PSUM. xp for the next s-phase split
    3:5 between DVE and GpSimd, emitted one block late to avoid
    head-of-line stalls in the in-order vector queue.
  - a(k), xp, and s(k+1) software-pipelined per n-block (skew 2) to keep
    PE fed; priors are never materialized. Input DMAs ordered so it0 and
    round 0 are not gated on late-arriving weights.

Engine partition bases: matmul operands and multi-input DVE/Pool ops at
base 0; single-input scalar.copy moves between 32-aligned bases. GpSimd
never touches PSUM. Layouts (i-major): chunk ch in [0,72): i = ch//9,
nb = ch%9, partition = n%128. (c,b) packed cb = 32*c + b.
"""

import numpy as np
import ml_dtypes

C, N, DIN, DOUT, B = 10, 1152, 8, 16, 256
NCORES, BL = 8, B // 8
NI = N * DIN          # 9216
NB = N // 128         # 9
NCH = DIN * NB        # 72
TW = C * BL           # 320
CD = C * DOUT         # 160

_PROG = None


def _build_program(stage=6):
    import concourse.bacc as bacc
    import concourse.tile as tile
    from concourse import mybir

    f32 = mybir.dt.float32
    bf16 = mybir.dt.bfloat16
    OP = mybir.AluOpType
    AF = mybir.ActivationFunctionType

    nc = bacc.Bacc("TRN2", target_bir_lowering=False, debug=False,
                   enable_asserts=False, num_devices=NCORES)

    WSP = 8
    wpc = (NCH // WSP) * CD
    xin_d = nc.dram_tensor("xin", [128, NCH * BL], bf16,
                           kind="ExternalInput").ap()
    xrep_d = nc.dram_tensor("xrep", [128, NCH * TW], bf16,
                            kind="ExternalInput").ap()
    ws_d = nc.dram_tensor("ws", [128, NCH * CD], bf16,
                          kind="ExternalInput").ap()
    wt4a_d = nc.dram_tensor("wt4a", [128, NI], bf16,
                            kind="ExternalInput").ap()
    wt4b_d = nc.dram_tensor("wt4b", [32, NI], bf16,
                            kind="ExternalInput").ap()
    ident_d = nc.dram_tensor("ident", [64, 64], f32,
                             kind="ExternalInput").ap()
    mask_d = nc.dram_tensor("mask", [32, TW], f32,
                            kind="ExternalInput").ap()
    vout_d = nc.dram_tensor("vout", [32, TW], f32,
                            kind="ExternalOutput").ap()

    with tile.TileContext(nc) as tc:
        with (
            tc.tile_pool(name="const", bufs=1) as cp,
            tc.tile_pool(name="xpb", bufs=6) as xpp,
            tc.tile_pool(name="tsl", bufs=3) as tpp,
            tc.tile_pool(name="vt", bufs=2) as vp,
            tc.tile_pool(name="pssa", bufs=1, space="PSUM") as pssa,
            tc.tile_pool(name="pssb", bufs=1, space="PSUM") as pssb,
            tc.tile_pool(name="psw", bufs=5, space="PSUM") as psw,
            tc.tile_pool(name="pzr", bufs=1, space="PSUM") as pzr,
        ):
            x_sb = cp.tile([128, NCH * BL], bf16)
            xrep = cp.tile([128, NCH * TW], bf16)
            wsp = [cp.tile([128, wpc], bf16, tag=f"ws{p}", name=f"ws{p}")
                   for p in range(WSP)]
            wt4a = cp.tile([128, NI], bf16)
            wt4b = cp.tile([32, NI], bf16)
            ident = cp.tile([64, 64], f32)
            mask = cp.tile([32, TW], f32)
            etb = cp.tile([128, NB * TW], bf16)
            Lbb = cp.tile([128, NB * TW], bf16)
            bdV01 = cp.tile([128, 256], bf16)
            bdV2 = cp.tile([32, 64], bf16)
            ones128 = cp.tile([128, 32], bf16)
            ones32 = cp.tile([32, 32], bf16)

            nc.sync.dma_start(x_sb[:], xin_d[:])
            for p_ in range(WSP):
                nc.sync.dma_start(wsp[p_][:], ws_d[:, wpc * p_:wpc * (p_ + 1)])
            nc.sync.dma_start(wt4b[:], wt4b_d[:])
            for h in range(2):
                nc.sync.dma_start(wt4a[:, NI // 2 * h:NI // 2 * (h + 1)],
                                  wt4a_d[:, NI // 2 * h:NI // 2 * (h + 1)])
            nc.sync.dma_start(ident[:], ident_d[:])
            nc.sync.dma_start(mask[:], mask_d[:])
            # xrep is nb-major: slab nb = [128, DIN*TW] feeds a_block(nb)
            xsl = DIN * TW
            for nb_ in range(NB):
                nc.sync.dma_start(xrep[:, xsl * nb_:xsl * (nb_ + 1)],
                                  xrep_d[:, xsl * nb_:xsl * (nb_ + 1)])
            nc.vector.memset(bdV01[:].bitcast(mybir.dt.uint32), 0)
            nc.vector.memset(bdV2[:].bitcast(mybir.dt.uint32), 0)
            nc.vector.memset(ones128[:].bitcast(mybir.dt.uint32), 0x3F803F80)
            nc.vector.memset(ones32[:].bitcast(mybir.dt.uint32), 0x3F803F80)

            def ws_chunk(ch, a, b):
                p_ = ch // (NCH // WSP)
                base = CD * (ch - p_ * (NCH // WSP))
                return wsp[p_][:, base + a:base + b]

            # scr2 pair layout [32, TW]: pair p (capsules 2p, 2p+1) occupies
            # cols 64p:64p+64; rows 0:16 <-> c=2p (block cols 0:32),
            # rows 16:32 <-> c=2p+1 (block cols 32:64). mask zeroes the rest.
            def squash(it, fill_scr2, zinfo, need_bdv, final):
                scr2 = vp.tile([32, TW], f32, tag="scr2")
                fill_scr2(scr2)
                s2m = vp.tile([32, TW], f32, tag="s2m")
                nc.vector.tensor_tensor(out=s2m[:], in0=scr2[:], in1=mask[:],
                                        op=OP.mult)
                sq = vp.tile([32, TW], bf16, tag="sq")
                nc.vector.tensor_tensor(out=sq[:], in0=s2m[:], in1=scr2[:],
                                        op=OP.mult)
                # norms: reduce over 32 partitions AND replicate to all rows
                # in one all-ones matmul.
                qsr = psw.tile([32, TW], f32, tag="wv", name=f"qsr{it}")
                nc.tensor.matmul(qsr[:], ones32[:], sq[:],
                                 start=True, stop=True)
                d = vp.tile([32, TW], f32, tag="d")
                rt = vp.tile([32, TW], f32, tag="rt")
                rec = vp.tile([32, TW], f32, tag="rec")
                g = vp.tile([32, TW], f32, tag="g")
                # g = sqrt(qs) / (Z^2 + qs); it0: Z = N.
                if it == 0:
                    nc.vector.tensor_scalar_add(d[:], qsr[:], float(N) ** 2)
                else:
                    nc.vector.tensor_tensor(out=d[:], in0=qsr[:],
                                            in1=zinfo["Z2"][:], op=OP.add)
                nc.scalar.activation(rt[:], qsr[:], AF.Sqrt)
                nc.vector.reciprocal_approx_fast(rec[:], d[:])
                nc.gpsimd.tensor_tensor(out=g[:], in0=rt[:], in1=rec[:],
                                        op=OP.mult)
                v2 = vp.tile([32, TW], f32, tag="v2")
                nc.vector.tensor_tensor(out=v2[:], in0=s2m[:], in1=g[:],
                                        op=OP.mult)
                if need_bdv:
                    for p_ in range(4):
                        nc.scalar.copy(
                            bdV01[32 * p_:32 * (p_ + 1),
                                  64 * p_:64 * (p_ + 1)],
                            v2[:, 64 * p_:64 * (p_ + 1)])
                    nc.scalar.copy(bdV2[:], v2[:, 256:TW])
                if final:
                    nc.sync.dma_start(vout_d[:], v2[:])

            # -------- it0: uniform probs, x-stationary transposed matmul ----
            psT = psw.tile([BL, CD], f32, tag="wv", name="psT")
            for ch in range(NCH):
                nc.tensor.matmul(psT[:], x_sb[:, BL * ch:BL * (ch + 1)],
                                 ws_chunk(ch, 0, CD),
                                 start=(ch == 0), stop=(ch == NCH - 1))
            sT2 = vp.tile([64, CD], f32, tag="sT2")
            nc.scalar.copy(sT2[0:BL, :], psT[:])
            nc.scalar.copy(sT2[BL:64, :], psT[:])
            pscr2 = psw.tile([32, TW], f32, tag="wv", name="pscr2")
            for p_ in range(5):
                nc.tensor.transpose(pscr2[0:32, 64 * p_:64 * (p_ + 1)],
                                    sT2[:, 32 * p_:32 * (p_ + 1)], ident[:])

            def fill0(scr2):
                nc.scalar.copy(scr2[:], pscr2[:])

            squash(0, fill0, None, need_bdv=True, final=(stage < 4))

            # -------- rounds: a(k) + exp + Z + s(k+1), pipelined ------------
            def round_(k, nbmax=NB):
                psa = pssa.tile([128, 8 * BL], f32, tag="psa")
                psb = pssb.tile([32, 2 * BL], f32, tag="psb")
                Zr = pzr.tile([32, TW], f32, tag="zr")
                xpbs = {}
                T1, T2, T4 = TW, 2 * TW, 4 * TW

                def gp_add(out, in0, in1):
                    nc.gpsimd.tensor_tensor(out=out, in0=in0, in1=in1,
                                            op=OP.add)

                def a_block(nb):
                    tsl = tpp.tile([128, DIN * TW], bf16, tag="t")
                    for i in range(DIN):
                        ch = i * NB + nb
                        pwv = psw.tile([128, TW], f32, tag="wv")
                        nc.tensor.matmul(pwv[:, 0:256],
                                         wt4a[:, 128 * ch:128 * (ch + 1)],
                                         bdV01[:], start=True, stop=True)
                        nc.tensor.matmul(pwv[:, 256:320],
                                         wt4b[0:32, 128 * ch:128 * (ch + 1)],
                                         bdV2[:], start=True, stop=True)
                        nc.vector.tensor_tensor(
                            out=tsl[:, TW * i:TW * (i + 1)],
                            in0=pwv[:],
                            in1=xrep[:, TW * (nb * DIN + i):
                                     TW * (nb * DIN + i) + TW],
                            op=OP.mult)
                    nc.vector.tensor_tensor(out=tsl[:, 0:T4],
                                            in0=tsl[:, 0:T4],
                                            in1=tsl[:, T4:2 * T4], op=OP.add)
                    gp_add(tsl[:, 0:T2], tsl[:, 0:T2], tsl[:, T2:T4])
                    Ls = Lbb[:, TW * nb:TW * (nb + 1)]
                    es = etb[:, TW * nb:TW * (nb + 1)]
                    if k == 0:
                        gp_add(Ls, tsl[:, 0:T1], tsl[:, T1:T2])
                    else:
                        gp_add(tsl[:, 0:T1], tsl[:, 0:T1], tsl[:, T1:T2])
                        gp_add(Ls, Ls, tsl[:, 0:T1])
                    nc.scalar.activation(es, Ls, AF.Exp)
                    nc.tensor.matmul(Zr[:], ones128[:], es,
                                     start=(nb == 0), stop=(nb == nbmax - 1))

                def xp_block(nb):
                    es = etb[:, TW * nb:TW * (nb + 1)]
                    xpb = xpp.tile([128, DIN * TW], bf16, tag="xp")
                    for i in range(DIN):
                        eng = nc.vector if i < 3 else nc.gpsimd
                        eng.tensor_tensor(
                            out=xpb[:, TW * i:TW * (i + 1)],
                            in0=xrep[:, TW * (nb * DIN + i):
                                     TW * (nb * DIN + i) + TW],
                            in1=es, op=OP.mult)
                    xpbs[nb] = xpb

                def s_block(nb):
                    xpb = xpbs[nb]
                    for i in range(DIN):
                        ch = i * NB + nb
                        first = (nb == 0 and i == 0)
                        last = (nb == NB - 1 and i == DIN - 1)
                        nc.tensor.matmul(psa[:], ws_chunk(ch, 0, 128),
                                         xpb[:, TW * i:TW * i + 256],
                                         start=first, stop=last)
                        nc.tensor.matmul(psb[:], ws_chunk(ch, 128, CD),
                                         xpb[:, TW * i + 256:TW * i + 320],
                                         start=first, stop=last)

                SKEW = 2
                for step in range(NB + SKEW):
                    if 1 <= step < nbmax + 1:
                        xp_block(step - 1)
                    if step < nbmax:
                        a_block(step)
                    if SKEW <= step < nbmax + SKEW:
                        s_block(step - SKEW)

                Z2 = vp.tile([32, TW], f32, tag="Z2")
                nc.scalar.activation(Z2[:], Zr[:], AF.Square)

                def fill(scr2):
                    for p_ in range(4):
                        nc.scalar.copy(scr2[:, 64 * p_:64 * (p_ + 1)],
                                       psa[32 * p_:32 * (p_ + 1),
                                           64 * p_:64 * (p_ + 1)])
                    nc.scalar.copy(scr2[:, 256:TW], psb[:])

                return {"Z2": Z2}, fill

            if stage >= 4:
                z1, fill1 = round_(0, nbmax=(NB if stage >= 5 else 1))
                if stage >= 6:
                    squash(1, fill1, z1, need_bdv=True, final=False)
                    z2, fill2 = round_(1)
                    squash(2, fill2, z2, need_bdv=False, final=True)
                else:
                    squash(1, fill1, z1, need_bdv=False, final=True)

    nc.compile()
    return nc


def _get_prog():
    global _PROG
    if _PROG is None:
        _PROG = _build_program()
    return _PROG


def _host_inputs(x, W):
    bf = ml_dtypes.bfloat16
    xf = np.ascontiguousarray(x, dtype=np.float32)
    Wf = np.ascontiguousarray(W, dtype=np.float32)
    ws = (Wf.transpose(2, 1, 0, 3)
          .reshape(DIN, NB, 128, C, DOUT)
          .transpose(2, 0, 1, 3, 4)
          .reshape(128, NCH * CD)).astype(bf)
    a4 = Wf.transpose(0, 3, 2, 1).reshape(C, DOUT, NI)    # [c, o, (i, n)]
    wt4a = np.concatenate(
        [a4[0:4].reshape(64, NI), a4[4:8].reshape(64, NI)], axis=0).astype(bf)
    wt4b = a4[8:10].reshape(32, NI).astype(bf)
    ident = np.eye(64, dtype=np.float32)
    mask = np.zeros((32, TW), dtype=np.float32)
    for col in range(TW):
        h = (col // 32) % 2
        mask[16 * h:16 * (h + 1), col] = 1.0
    maps = []
    for k in range(NCORES):
        xs = (xf[BL * k:BL * (k + 1)]
              .transpose(2, 1, 0)
              .reshape(DIN, NB, 128, BL)
              .transpose(2, 0, 1, 3)
              .reshape(128, NCH * BL)).astype(bf)
        # xrep: [p, (nb, i, c, b)] = xs broadcast over c, nb-major
        xr = (np.broadcast_to(xs.reshape(128, DIN, NB, 1, BL),
                              (128, DIN, NB, C, BL))
              .transpose(0, 2, 1, 3, 4).reshape(128, NCH * TW))
        maps.append({
            "xin": np.ascontiguousarray(xs),
            "xrep": np.ascontiguousarray(xr),
            "ws": np.ascontiguousarray(ws),
            "wt4a": np.ascontiguousarray(wt4a),
            "wt4b": np.ascontiguousarray(wt4b),
            "ident": ident,
            "mask": mask,
        })
    return maps


def kernel(x, W):
    from concourse.bass_utils import run_bass_kernel_spmd
    nc = _get_prog()
    in_maps = _host_inputs(x, W)
    res = run_bass_kernel_spmd(nc, in_maps, core_ids=list(range(NCORES)))
    out = np.zeros((C, B, 1, DOUT), dtype=np.float32)
    for k in range(NCORES):
        vo = res.results[k]["vout"]              # [32, TW] pair layout
        for c_ in range(C):
            p_, h = c_ // 2, c_ % 2
            blk = vo[16 * h:16 * (h + 1),
                     64 * p_ + 32 * h:64 * p_ + 32 * (h + 1)]
            out[c_, BL * k:BL * (k + 1), 0, :] = blk.T
    return out


# revision 46
# speedup vs baseline: 1.2562x; 1.0311x over previous
"""Trainium2 Bass kernel for DigitCaps dynamic-routing layer (v6, bf16).

priors[c,b,n,o] = sum_i x[b,n,i] * W[c,n,i,o]; 3 softmax-routing iterations.
Output: squash(sum_n probs * priors) of the last iteration, [C,B,1,DOUT].

Strategy (B data-parallel over 8 cores, BL=32 per core, all on-chip, bf16):
  - it0 (uniform probs): s0^T[b,(c,o)] via x-stationary matmuls (160 cols per
    (n,i) chunk), then 5 capsule-pair transposes into the squash layout.
  - it1/2: s[(c,o),(c,b)] block-diag via ws-stationary matmuls, exp-weights
    folded into x (xp bf16); moving columns split 256/64 between the two
    (c,o)-row groups.
  - squash in a capsule-pair layout scr2[32, 320]: aligned scalar-engine
    copies of [32,64] pair blocks from PSUM (junk killed by a constant
    mask); norms AND Z reduced+replicated by all-ones bf16 matmuls;
    divide-free scale g = sqrt(qs)/(Z^2 + qs); v = s*g. The zeroed pair
    blocks double as the block-diag V operand of the next agreement phase.
  - agreement a[c,b,n] = sum_i x * (W v): Wv on PE, x-mult as flat DVE ops
    against a host-replicated nb-major xrep (no broadcast views),
    i-reduction as a bf16 tree split across DVE and GpSimd; Z accumulated
    per n-block by ones-matmuls into PSUM. xp for the next s-phase split
    3:5 between DVE and GpSimd, emitted one block late to avoid
    head-of-line stalls in the in-order vector queue.
  - a(k), xp, and s(k+1) software-pipelined per n-block (skew 2) to keep
    PE fed; priors are never materialized. Input DMAs ordered so it0 and
    round 0 are not gated on late-arriving weights.

Engine partition bases: matmul operands and multi-input DVE/Pool ops at
base 0; single-input scalar.copy moves between 32-aligned bases. GpSimd
never touches PSUM. Layouts (i-major): chunk ch in [0,72): i = ch//9,
nb = ch%9, partition = n%128. (c,b) packed cb = 32*c + b.
"""

import numpy as np
import ml_dtypes

C, N, DIN, DOUT, B = 10, 1152, 8, 16, 256
NCORES, BL = 8, B // 8
NI = N * DIN          # 9216
NB = N // 128         # 9
NCH = DIN * NB        # 72
TW = C * BL           # 320
CD = C * DOUT         # 160

_PROG = None


def _build_program(stage=6):
    import concourse.bacc as bacc
    import concourse.tile as tile
    from concourse import mybir

    f32 = mybir.dt.float32
    bf16 = mybir.dt.bfloat16
    OP = mybir.AluOpType
    AF = mybir.ActivationFunctionType

    nc = bacc.Bacc("TRN2", target_bir_lowering=False, debug=False,
                   enable_asserts=False, num_devices=NCORES)

    WSP = 8
    wpc = (NCH // WSP) * CD
    xin_d = nc.dram_tensor("xin", [128, NCH * BL], bf16,
                           kind="ExternalInput").ap()
    xrep_d = nc.dram_tensor("xrep", [128, NCH * TW], bf16,
                            kind="ExternalInput").ap()
    ws_d = nc.dram_tensor("ws", [128, NCH * CD], bf16,
                          kind="ExternalInput").ap()
    wt4a_d = nc.dram_tensor("wt4a", [128, NI], bf16,
                            kind="ExternalInput").ap()
    wt4b_d = nc.dram_tensor("wt4b", [32, NI], bf16,
                            kind="ExternalInput").ap()
    ident_d = nc.dram_tensor("ident", [64, 64], f32,
                             kind="ExternalInput").ap()
    mask_d = nc.dram_tensor("mask", [32, TW], f32,
                            kind="ExternalInput").ap()
    vout_d = nc.dram_tensor("vout", [32, TW], f32,
                            kind="ExternalOutput").ap()

    with tile.TileContext(nc) as tc:
        with (
            tc.tile_pool(name="const", bufs=1) as cp,
            tc.tile_pool(name="xpb", bufs=6) as xpp,
            tc.tile_pool(name="tsl", bufs=3) as tpp,
            tc.tile_pool(name="vt", bufs=2) as vp,
            tc.tile_pool(name="pssa", bufs=1, space="PSUM") as pssa,
            tc.tile_pool(name="pssb", bufs=1, space="PSUM") as pssb,
            tc.tile_pool(name="psw", bufs=5, space="PSUM") as psw,
            tc.tile_pool(name="pzr", bufs=1, space="PSUM") as pzr,
        ):
            x_sb = cp.tile([128, NCH * BL], bf16)
            xrep = cp.tile([128, NCH * TW], bf16)
            wsp = [cp.tile([128, wpc], bf16, tag=f"ws{p}", name=f"ws{p}")
                   for p in range(WSP)]
            wt4a = cp.tile([128, NI], bf16)
            wt4b = cp.tile([32, NI], bf16)
            ident = cp.tile([64, 64], f32)
            mask = cp.tile([32, TW], f32)
            etb = cp.tile([128, NB * TW], bf16)
            Lbb = cp.tile([128, NB * TW], bf16)
            bdV01 = cp.tile([128, 256], bf16)
            bdV2 = cp.tile([32, 64], bf16)
            ones128 = cp.tile([128, 32], bf16)
            ones32 = cp.tile([32, 32], bf16)

            nc.sync.dma_start(x_sb[:], xin_d[:])
            nc.sync.dma_start(ident[:], ident_d[:])
            nc.sync.dma_start(mask[:], mask_d[:])
            for p_ in range(WSP):
                nc.sync.dma_start(wsp[p_][:], ws_d[:, wpc * p_:wpc * (p_ + 1)])
            nc.sync.dma_start(wt4b[:], wt4b_d[:])
            for h in range(2):
                nc.sync.dma_start(wt4a[:, NI // 2 * h:NI // 2 * (h + 1)],
                                  wt4a_d[:, NI // 2 * h:NI // 2 * (h + 1)])
            # xrep is nb-major: slab nb = [128, DIN*TW] feeds a_block(nb)
            xsl = DIN * TW
            for nb_ in range(NB):
                nc.sync.dma_start(xrep[:, xsl * nb_:xsl * (nb_ + 1)],
                                  xrep_d[:, xsl * nb_:xsl * (nb_ + 1)])
            nc.vector.memset(bdV01[:].bitcast(mybir.dt.uint32), 0)
            nc.vector.memset(bdV2[:].bitcast(mybir.dt.uint32), 0)
            nc.vector.memset(ones128[:].bitcast(mybir.dt.uint32), 0x3F803F80)
            nc.vector.memset(ones32[:].bitcast(mybir.dt.uint32), 0x3F803F80)

            def ws_chunk(ch, a, b):
                p_ = ch // (NCH // WSP)
                base = CD * (ch - p_ * (NCH // WSP))
                return wsp[p_][:, base + a:base + b]

            # scr2 pair layout [32, TW]: pair p (capsules 2p, 2p+1) occupies
            # cols 64p:64p+64; rows 0:16 <-> c=2p (block cols 0:32),
            # rows 16:32 <-> c=2p+1 (block cols 32:64). mask zeroes the rest.
            def squash(it, fill_scr2, zinfo, need_bdv, final):
                scr2 = vp.tile([32, TW], f32, tag="scr2")
                fill_scr2(scr2)
                s2m = vp.tile([32, TW], f32, tag="s2m")
                nc.vector.tensor_tensor(out=s2m[:], in0=scr2[:], in1=mask[:],
                                        op=OP.mult)
                sq = vp.tile([32, TW], bf16, tag="sq")
                nc.vector.tensor_tensor(out=sq[:], in0=s2m[:], in1=scr2[:],
                                        op=OP.mult)
                # norms: reduce over 32 partitions AND replicate to all rows
                # in one all-ones matmul.
                qsr = psw.tile([32, TW], f32, tag="wv", name=f"qsr{it}")
                nc.tensor.matmul(qsr[:], ones32[:], sq[:],
                                 start=True, stop=True)
                d = vp.tile([32, TW], f32, tag="d")
                rt = vp.tile([32, TW], f32, tag="rt")
                rec = vp.tile([32, TW], f32, tag="rec")
                g = vp.tile([32, TW], f32, tag="g")
                # g = sqrt(qs) / (Z^2 + qs); it0: Z = N.
                if it == 0:
                    nc.vector.tensor_scalar_add(d[:], qsr[:], float(N) ** 2)
                else:
                    nc.vector.tensor_tensor(out=d[:], in0=qsr[:],
                                            in1=zinfo["Z2"][:], op=OP.add)
                nc.scalar.activation(rt[:], qsr[:], AF.Sqrt)
                nc.vector.reciprocal_approx_fast(rec[:], d[:])
                nc.gpsimd.tensor_tensor(out=g[:], in0=rt[:], in1=rec[:],
                                        op=OP.mult)
                v2 = vp.tile([32, TW], f32, tag="v2")
                nc.vector.tensor_tensor(out=v2[:], in0=s2m[:], in1=g[:],
                                        op=OP.mult)
                if need_bdv:
                    for p_ in range(4):
                        nc.scalar.copy(
                            bdV01[32 * p_:32 * (p_ + 1),
                                  64 * p_:64 * (p_ + 1)],
                            v2[:, 64 * p_:64 * (p_ + 1)])
                    nc.scalar.copy(bdV2[:], v2[:, 256:TW])
                if final:
                    nc.sync.dma_start(vout_d[:], v2[:])

            # -------- it0: uniform probs, x-stationary transposed matmul ----
            psT = psw.tile([BL, CD], f32, tag="wv", name="psT")
            for ch in range(NCH):
                nc.tensor.matmul(psT[:], x_sb[:, BL * ch:BL * (ch + 1)],
                                 ws_chunk(ch, 0, CD),
                                 start=(ch == 0), stop=(ch == NCH - 1))
            sT2 = vp.tile([64, CD], f32, tag="sT2")
            nc.scalar.copy(sT2[0:BL, :], psT[:])
            nc.scalar.copy(sT2[BL:64, :], psT[:])
            pscr2 = psw.tile([32, TW], f32, tag="wv", name="pscr2")
            for p_ in range(5):
                nc.tensor.transpose(pscr2[0:32, 64 * p_:64 * (p_ + 1)],
                                    sT2[:, 32 * p_:32 * (p_ + 1)], ident[:])

            def fill0(scr2):
                nc.scalar.copy(scr2[:], pscr2[:])

            squash(0, fill0, None, need_bdv=True, final=(stage < 4))

            # -------- rounds: a(k) + exp + Z + s(k+1), pipelined ------------
            def round_(k, nbmax=NB):
                psa = pssa.tile([128, 8 * BL], f32, tag="psa")
                psb = pssb.tile([32, 2 * BL], f32, tag="psb")
                Zr = pzr.tile([32, TW], f32, tag="zr")
                xpbs = {}
                T1, T2, T4 = TW, 2 * TW, 4 * TW

                def gp_add(out, in0, in1):
                    nc.gpsimd.tensor_tensor(out=out, in0=in0, in1=in1,
                                            op=OP.add)

                def a_block(nb):
                    tsl = tpp.tile([128, DIN * TW], bf16, tag="t")
                    for i in range(DIN):
                        ch = i * NB + nb
                        pwv = psw.tile([128, TW], f32, tag="wv")
                        nc.tensor.matmul(pwv[:, 0:256],
                                         wt4a[:, 128 * ch:128 * (ch + 1)],
                                         bdV01[:], start=True, stop=True)
                        nc.tensor.matmul(pwv[:, 256:320],
                                         wt4b[0:32, 128 * ch:128 * (ch + 1)],
                                         bdV2[:], start=True, stop=True)
                        nc.vector.tensor_tensor(
                            out=tsl[:, TW * i:TW * (i + 1)],
                            in0=pwv[:],
                            in1=xrep[:, TW * (nb * DIN + i):
                                     TW * (nb * DIN + i) + TW],
                            op=OP.mult)
                    nc.vector.tensor_tensor(out=tsl[:, 0:T4],
                                            in0=tsl[:, 0:T4],
                                            in1=tsl[:, T4:2 * T4], op=OP.add)
                    gp_add(tsl[:, 0:T2], tsl[:, 0:T2], tsl[:, T2:T4])
                    Ls = Lbb[:, TW * nb:TW * (nb + 1)]
                    es = etb[:, TW * nb:TW * (nb + 1)]
                    if k == 0:
                        gp_add(Ls, tsl[:, 0:T1], tsl[:, T1:T2])
                    else:
                        gp_add(tsl[:, 0:T1], tsl[:, 0:T1], tsl[:, T1:T2])
                        gp_add(Ls, Ls, tsl[:, 0:T1])
                    nc.scalar.activation(es, Ls, AF.Exp)
                    nc.tensor.matmul(Zr[:], ones128[:], es,
                                     start=(nb == 0), stop=(nb == nbmax - 1))

                def xp_block(nb):
                    es = etb[:, TW * nb:TW * (nb + 1)]
                    xpb = xpp.tile([128, DIN * TW], bf16, tag="xp")
                    for i in range(DIN):
                        eng = nc.vector if i < 3 else nc.gpsimd
                        eng.tensor_tensor(
                            out=xpb[:, TW * i:TW * (i + 1)],
                            in0=xrep[:, TW * (nb * DIN + i):
                                     TW * (nb * DIN + i) + TW],
                            in1=es, op=OP.mult)
                    xpbs[nb] = xpb

                def s_block(nb):
                    xpb = xpbs[nb]
                    for i in range(DIN):
                        ch = i * NB + nb
                        first = (nb == 0 and i == 0)
                        last = (nb == NB - 1 and i == DIN - 1)
                        nc.tensor.matmul(psa[:], ws_chunk(ch, 0, 128),
                                         xpb[:, TW * i:TW * i + 256],
                                         start=first, stop=last)
                        nc.tensor.matmul(psb[:], ws_chunk(ch, 128, CD),
                                         xpb[:, TW * i + 256:TW * i + 320],
                                         start=first, stop=last)

                SKEW = 2
                for step in range(NB + SKEW):
                    if 1 <= step < nbmax + 1:
                        xp_block(step - 1)
                    if step < nbmax:
                        a_block(step)
                    if SKEW <= step < nbmax + SKEW:
                        s_block(step - SKEW)

                Z2 = vp.tile([32, TW], f32, tag="Z2")
                nc.scalar.activation(Z2[:], Zr[:], AF.Square)

                def fill(scr2):
                    for p_ in range(4):
                        nc.scalar.copy(scr2[:, 64 * p_:64 * (p_ + 1)],
                                       psa[32 * p_:32 * (p_ + 1),
                                           64 * p_:64 * (p_ + 1)])
                    nc.scalar.copy(scr2[:, 256:TW], psb[:])

                return {"Z2": Z2}, fill

            if stage >= 4:
                z1, fill1 = round_(0, nbmax=(NB if stage >= 5 else 1))
                if stage >= 6:
                    squash(1, fill1, z1, need_bdv=True, final=False)
                    z2, fill2 = round_(1)
                    squash(2, fill2, z2, need_bdv=False, final=True)
                else:
                    squash(1, fill1, z1, need_bdv=False, final=True)

    nc.compile()
    return nc


def _get_prog():
    global _PROG
    if _PROG is None:
        _PROG = _build_program()
    return _PROG


def _host_inputs(x, W):
    bf = ml_dtypes.bfloat16
    xf = np.ascontiguousarray(x, dtype=np.float32)
    Wf = np.ascontiguousarray(W, dtype=np.float32)
    ws = (Wf.transpose(2, 1, 0, 3)
          .reshape(DIN, NB, 128, C, DOUT)
          .transpose(2, 0, 1, 3, 4)
          .reshape(128, NCH * CD)).astype(bf)
    a4 = Wf.transpose(0, 3, 2, 1).reshape(C, DOUT, NI)    # [c, o, (i, n)]
    wt4a = np.concatenate(
        [a4[0:4].reshape(64, NI), a4[4:8].reshape(64, NI)], axis=0).astype(bf)
    wt4b = a4[8:10].reshape(32, NI).astype(bf)
    ident = np.eye(64, dtype=np.float32)
    mask = np.zeros((32, TW), dtype=np.float32)
    for col in range(TW):
        h = (col // 32) % 2
        mask[16 * h:16 * (h + 1), col] = 1.0
    maps = []
    for k in range(NCORES):
        xs = (xf[BL * k:BL * (k + 1)]
              .transpose(2, 1, 0)
              .reshape(DIN, NB, 128, BL)
              .transpose(2, 0, 1, 3)
              .reshape(128, NCH * BL)).astype(bf)
        # xrep: [p, (nb, i, c, b)] = xs broadcast over c, nb-major
        xr = (np.broadcast_to(xs.reshape(128, DIN, NB, 1, BL),
                              (128, DIN, NB, C, BL))
              .transpose(0, 2, 1, 3, 4).reshape(128, NCH * TW))
        maps.append({
            "xin": np.ascontiguousarray(xs),
            "xrep": np.ascontiguousarray(xr),
            "ws": np.ascontiguousarray(ws),
            "wt4a": np.ascontiguousarray(wt4a),
            "wt4b": np.ascontiguousarray(wt4b),
            "ident": ident,
            "mask": mask,
        })
    return maps


def kernel(x, W):
    from concourse.bass_utils import run_bass_kernel_spmd
    nc = _get_prog()
    in_maps = _host_inputs(x, W)
    res = run_bass_kernel_spmd(nc, in_maps, core_ids=list(range(NCORES)))
    out = np.zeros((C, B, 1, DOUT), dtype=np.float32)
    for k in range(NCORES):
        vo = res.results[k]["vout"]              # [32, TW] pair layout
        for c_ in range(C):
            p_, h = c_ // 2, c_ % 2
            blk = vo[16 * h:16 * (h + 1),
                     64 * p_ + 32 * h:64 * p_ + 32 * (h + 1)]
            out[c_, BL * k:BL * (k + 1), 0, :] = blk.T
    return out
